# revision 1
# baseline (speedup 1.0000x reference)
"""CTC loss (Keras ctc_batch_cost semantics) on 8 Trainium2 NeuronCores.

Strategy: pure data parallelism — batch B=1024 sharded 128/core (batch =
SBUF partition dim). Host does index preparation only (extended-label
gather of y_pred, skip masks, readout mask, reachability mask); each core
runs the CTC lattice sum in the linear-probability domain, split into a
forward DP (t=0..127) and a backward suffix DP (t=255..128) whose chains
are interleaved on the Vector engine. Every 8 steps each chain is
renormalized to 2^48 by its row max (scale logs accumulated via the Scalar
engine's Ln). The two halves meet in a one-time log-domain seam:
ll = m + ln Σ_s exp(ln α + ln B − m) + Σ ln(scales). No collectives; host
concatenates the per-core [128,1] outputs.
"""

import numpy as np

import concourse.bacc as bacc
import concourse.mybir as mybir
import concourse.tile as tile
from concourse.bass_utils import run_bass_kernel_spmd

B, T, C, U = 1024, 256, 100, 48
S = 2 * U + 1          # 97 extended-label positions
SP = 104               # S padded so each (b,t) row is 416B (32B aligned)
BLANK = C - 1
EPS = 1e-7
NCORES = 8
BS = B // NCORES       # 128 samples per core = SBUF partition dim
CH = 16                # time steps per DMA chunk
NCH = T // CH
RENORM = 8
TM = 127               # forward covers t=1..TM; backward t=T-1..TM+1
RT_LOG2 = 48           # renorm target 2^48
SEAM_LOG2 = 40         # per-sample seam scale target 2^40 (power of 2)
NREN_F = len([t for t in range(1, TM + 1) if t % RENORM == 0])          # 15
NREN_B = len([t for t in range(T - 1, TM, -1) if (T - 1 - t) % RENORM == RENORM - 1])  # 16
NLG = NREN_F + NREN_B + 2  # +2 seam-side exponent shifts; all in ln2 units
F32 = mybir.dt.float32
ALU = mybir.AluOpType
AXX = mybir.AxisListType.X
ACTF = mybir.ActivationFunctionType
LN2 = float(np.log(2.0))
I32 = mybir.dt.int32
TINY = 1e-38
DEAD = -1000.0


def _emit(nc, tc, p_d, skip_d, skip2_d, sel_d, out_d):
    v = nc.vector
    with tc.tile_pool(name="pchunks", bufs=1) as ppool, tc.tile_pool(
        name="work", bufs=1
    ) as wp:
        skip_t = wp.tile([BS, SP], F32, name="skip_t")
        nc.sync.dma_start(out=skip_t[:], in_=skip_d)
        skip2_t = wp.tile([BS, SP], F32, name="skip2_t")
        nc.sync.dma_start(out=skip2_t[:], in_=skip2_d)
        sel_t = wp.tile([BS, SP], F32, name="sel_t")
        nc.sync.dma_start(out=sel_t[:], in_=sel_d)
        # forward eats chunks 0.. up; backward eats 15.. down — interleave
        # the loads so both chains can start immediately.
        pts = [None] * NCH
        order = []
        for i in range(NCH // 2):
            order += [i, NCH - 1 - i]
        for ci in order:
            pch = ppool.tile([BS, CH * SP], F32, name=f"pch{ci}")
            nc.sync.dma_start(out=pch[:], in_=p_d[:, ci * CH : (ci + 1) * CH, :])
            pts[ci] = pch

        # forward state: 2 permanently-zero cols in FRONT (s-1/s-2 reads).
        aA = wp.tile([BS, SP + 2], F32, name="aA")
        aB = wp.tile([BS, SP + 2], F32, name="aB")
        # backward state + q scratch: zero cols at the END (s+1/s+2 reads).
        bA = wp.tile([BS, SP + 2], F32, name="bA")
        bB = wp.tile([BS, SP + 2], F32, name="bB")
        q_t = wp.tile([BS, S + 2], F32, name="q_t")
        wf = wp.tile([BS, S], F32, name="wf")
        vf = wp.tile([BS, S], F32, name="vf")
        uf = wp.tile([BS, S], F32, name="uf")
        wb = wp.tile([BS, S], F32, name="wb")
        vb = wp.tile([BS, S], F32, name="vb")
        rf0 = wp.tile([BS, 1], F32, name="rf0")
        fe1 = wp.tile([BS, 1], I32, name="fe1")
        fe2 = wp.tile([BS, 1], I32, name="fe2")
        fe3 = wp.tile([BS, 1], I32, name="fe3")
        rb0 = wp.tile([BS, 1], F32, name="rb0")
        be1 = wp.tile([BS, 1], I32, name="be1")
        be2 = wp.tile([BS, 1], I32, name="be2")
        be3 = wp.tile([BS, 1], I32, name="be3")
        lgbuf = wp.tile([BS, NLG], F32, name="lgbuf")

        v.memset(aA[:], 0.0)
        v.memset(aB[:], 0.0)
        v.memset(bA[:], 0.0)
        v.memset(bB[:], 0.0)
        v.memset(q_t[:], 0.0)
        # alpha_0: s=0 (blank) and s=1 (first label); p chunk 0 cols 0:2.
        v.tensor_copy(out=aA[:, 2:4], in_=pts[0][:, 0:2])
        # B_{T-1} = sel
        v.tensor_copy(out=bA[:, 0:SP], in_=sel_t[:])

        def fwd_step(t, cur, nxt, k):
            ci, off = t // CH, (t % CH) * SP
            pt = pts[ci][:, off : off + S]
            renorm = t % RENORM == 0
            if renorm:
                # power-of-2 renorm: rinv = 2^-(e(max)-48); e-shift -> lgbuf
                v.tensor_reduce(out=rf0[:], in_=cur[:, 2 : 2 + S], axis=AXX, op=ALU.max)
                v.tensor_scalar(out=fe1[:], in0=rf0[:].bitcast(I32), scalar1=23, scalar2=None, op0=ALU.logical_shift_right)
                v.tensor_scalar(out=fe1[:], in0=fe1[:], scalar1=127 + RT_LOG2, scalar2=None, op0=ALU.subtract)
                v.tensor_copy(out=lgbuf[:, k : k + 1], in_=fe1[:])
                v.tensor_scalar(out=fe2[:], in0=fe1[:], scalar1=-1, scalar2=127, op0=ALU.mult, op1=ALU.add)
                v.tensor_scalar(out=fe3[:], in0=fe2[:], scalar1=23, scalar2=None, op0=ALU.logical_shift_left)
            v.tensor_tensor(out=wf[:], in0=cur[:, 2 : 2 + S], in1=cur[:, 1 : 1 + S], op=ALU.add)
            v.tensor_tensor(out=vf[:], in0=cur[:, 0:S], in1=skip_t[:, 0:S], op=ALU.mult)
            v.tensor_tensor(out=uf[:], in0=wf[:], in1=vf[:], op=ALU.add)
            if renorm:
                v.scalar_tensor_tensor(
                    out=nxt[:, 2 : 2 + S], in0=uf[:], scalar=fe3[:].bitcast(F32)[:, 0:1], in1=pt,
                    op0=ALU.mult, op1=ALU.mult,
                )
                return 1
            v.tensor_tensor(out=nxt[:, 2 : 2 + S], in0=uf[:], in1=pt, op=ALU.mult)
            return 0

        def bwd_step(t, cur, nxt, k):
            # computes B_{t-1} from B_t;  q = P_t * B_t
            ci, off = t // CH, (t % CH) * SP
            pt = pts[ci][:, off : off + S]
            renorm = (T - 1 - t) % RENORM == RENORM - 1
            if renorm:
                v.tensor_reduce(out=rb0[:], in_=cur[:, 0:S], axis=AXX, op=ALU.max)
                v.tensor_scalar(out=be1[:], in0=rb0[:].bitcast(I32), scalar1=23, scalar2=None, op0=ALU.logical_shift_right)
                v.tensor_scalar(out=be1[:], in0=be1[:], scalar1=127 + RT_LOG2, scalar2=None, op0=ALU.subtract)
                v.tensor_copy(out=lgbuf[:, k : k + 1], in_=be1[:])
                v.tensor_scalar(out=be2[:], in0=be1[:], scalar1=-1, scalar2=127, op0=ALU.mult, op1=ALU.add)
                v.tensor_scalar(out=be3[:], in0=be2[:], scalar1=23, scalar2=None, op0=ALU.logical_shift_left)
                v.scalar_tensor_tensor(
                    out=q_t[:, 0:S], in0=cur[:, 0:S], scalar=be3[:].bitcast(F32)[:, 0:1], in1=pt,
                    op0=ALU.mult, op1=ALU.mult,
                )
            else:
                v.tensor_tensor(out=q_t[:, 0:S], in0=cur[:, 0:S], in1=pt, op=ALU.mult)
            v.tensor_tensor(out=wb[:], in0=q_t[:, 0:S], in1=q_t[:, 1 : 1 + S], op=ALU.add)
            v.tensor_tensor(out=vb[:], in0=q_t[:, 2 : 2 + S], in1=skip2_t[:, 0:S], op=ALU.mult)
            v.tensor_tensor(out=nxt[:, 0:S], in0=wb[:], in1=vb[:], op=ALU.add)
            return 1 if renorm else 0

        # interleave the two independent chains so the DVE never stalls on
        # a single chain's serial dependency.
        kf, kb = 0, NREN_F
        fa, fb_ = aA, aB
        ba, bb_ = bA, bB
        ts_f = list(range(1, TM + 1))          # 127 steps
        ts_b = list(range(T - 1, TM, -1))      # 128 steps
        for i in range(max(len(ts_f), len(ts_b))):
            if i < len(ts_f):
                kf += fwd_step(ts_f[i], fa, fb_, kf)
                fa, fb_ = fb_, fa
            if i < len(ts_b):
                kb += bwd_step(ts_b[i], ba, bb_, kb)
                ba, bb_ = bb_, ba
        # results: alpha_TM in fa, B_TM in ba
        assert kf == NREN_F and kb == NREN_F + NREN_B

        # ---- seam: ll = m + ln Σ exp(lnα + lnB − m) + ln2·Σ lgbuf ----
        # per-side power-of-2 scale to 2^40 + tiny floor, then an exact
        # bit-extracted log (LUT only sees mantissas in [1,2)).
        as_t = wp.tile([BS, S], F32, name="as_t")
        ei_t = wp.tile([BS, S], I32, name="ei_t")
        mi_t = wp.tile([BS, S], I32, name="mi_t")
        ef_t = wp.tile([BS, S], F32, name="ef_t")
        lm_t = wp.tile([BS, S], F32, name="lm_t")
        la = wp.tile([BS, S], F32, name="la")
        la2 = wp.tile([BS, S], F32, name="la2")
        lb2 = wp.tile([BS, S], F32, name="lb2")
        da = wp.tile([BS, S], F32, name="da")
        lam = wp.tile([BS, S], F32, name="lam")
        m_t = wp.tile([BS, 1], F32, name="m_t")
        nm_t = wp.tile([BS, 1], F32, name="nm_t")
        e_t = wp.tile([BS, S], F32, name="e_t")
        z_t = wp.tile([BS, 1], F32, name="z_t")
        lnz_t = wp.tile([BS, 1], F32, name="lnz_t")
        racc = wp.tile([BS, 1], F32, name="racc")
        acc1 = wp.tile([BS, 1], F32, name="acc1")
        acc2 = wp.tile([BS, 1], F32, name="acc2")
        outt = wp.tile([BS, 1], F32, name="outt")
        sm0 = wp.tile([BS, 1], F32, name="sm0")
        se1 = wp.tile([BS, 1], I32, name="se1")
        se2 = wp.tile([BS, 1], I32, name="se2")
        se3 = wp.tile([BS, 1], I32, name="se3")

        def side_log(x_ap, out_lam, lg_col):
            # seam scale s = 2^-(e(max)-40); e-shift into lgbuf[lg_col]
            v.tensor_reduce(out=sm0[:], in_=x_ap, axis=AXX, op=ALU.max)
            v.tensor_scalar(out=se1[:], in0=sm0[:].bitcast(I32), scalar1=23, scalar2=None, op0=ALU.logical_shift_right)
            v.tensor_scalar(out=se1[:], in0=se1[:], scalar1=127 + SEAM_LOG2, scalar2=None, op0=ALU.subtract)
            v.tensor_copy(out=lgbuf[:, lg_col : lg_col + 1], in_=se1[:])
            v.tensor_scalar(out=se2[:], in0=se1[:], scalar1=-1, scalar2=127, op0=ALU.mult, op1=ALU.add)
            v.tensor_scalar(out=se3[:], in0=se2[:], scalar1=23, scalar2=None, op0=ALU.logical_shift_left)
            # y = x*s + tiny, then exact log of y
            v.tensor_scalar(out=as_t[:], in0=x_ap, scalar1=se3[:].bitcast(F32)[:, 0:1], scalar2=TINY, op0=ALU.mult, op1=ALU.add)
            ai = as_t[:].bitcast(I32)
            v.tensor_scalar(out=ei_t[:], in0=ai, scalar1=23, scalar2=None, op0=ALU.logical_shift_right)
            v.tensor_scalar(out=ei_t[:], in0=ei_t[:], scalar1=127, scalar2=None, op0=ALU.subtract)
            v.tensor_scalar(out=mi_t[:], in0=ai, scalar1=0x007FFFFF, scalar2=0x3F800000, op0=ALU.bitwise_and, op1=ALU.bitwise_or)
            v.tensor_copy(out=ef_t[:], in_=ei_t[:])
            nc.scalar.activation(out=lm_t[:], in_=mi_t[:].bitcast(F32), func=ACTF.Ln)
            v.scalar_tensor_tensor(out=la[:], in0=ef_t[:], scalar=LN2, in1=lm_t[:], op0=ALU.mult, op1=ALU.add)
            v.tensor_scalar(out=da[:], in0=x_ap, scalar1=0.0, scalar2=None, op0=ALU.is_equal)
            v.scalar_tensor_tensor(out=out_lam[:], in0=da[:], scalar=DEAD, in1=la[:], op0=ALU.mult, op1=ALU.add)

        al = fa[:, 2 : 2 + S]
        bl = ba[:, 0:S]
        side_log(al, la2, NLG - 2)
        side_log(bl, lb2, NLG - 1)
        v.tensor_tensor(out=lam[:], in0=la2[:], in1=lb2[:], op=ALU.add)
        v.tensor_reduce(out=m_t[:], in_=lam[:], axis=AXX, op=ALU.max)
        v.tensor_scalar(out=nm_t[:], in0=m_t[:], scalar1=-1.0, scalar2=None, op0=ALU.mult)
        nc.scalar.activation(out=e_t[:], in_=lam[:], func=ACTF.Exp, bias=nm_t[:, 0:1], scale=1.0)
        v.tensor_reduce(out=z_t[:], in_=e_t[:], axis=AXX, op=ALU.add)
        nc.scalar.activation(out=lnz_t[:], in_=z_t[:], func=ACTF.Ln)
        v.tensor_reduce(out=racc[:], in_=lgbuf[:], axis=AXX, op=ALU.add)
        v.scalar_tensor_tensor(out=acc1[:], in0=racc[:], scalar=LN2, in1=m_t[:], op0=ALU.mult, op1=ALU.add)
        v.tensor_tensor(out=acc2[:], in0=acc1[:], in1=lnz_t[:], op=ALU.add)
        nc.scalar.mul(out=outt[:], in_=acc2[:], mul=-1.0)
        nc.sync.dma_start(out=out_d, in_=outt[:])


def _build_program():
    nc = bacc.Bacc("TRN2", target_bir_lowering=False, debug=False)
    p_d = nc.dram_tensor("p", [BS, T, SP], F32, kind="ExternalInput").ap()
    skip_d = nc.dram_tensor("skip", [BS, SP], F32, kind="ExternalInput").ap()
    skip2_d = nc.dram_tensor("skip2", [BS, SP], F32, kind="ExternalInput").ap()
    sel_d = nc.dram_tensor("sel", [BS, SP], F32, kind="ExternalInput").ap()
    out_d = nc.dram_tensor("out", [BS, 1], F32, kind="ExternalOutput").ap()
    with tile.TileContext(nc) as tc:
        _emit(nc, tc, p_d, skip_d, skip2_d, sel_d, out_d)
    nc.compile()
    return nc


_NC = None


def _get_nc():
    global _NC
    if _NC is None:
        _NC = _build_program()
    return _NC


def _prep_in_maps(y_pred, y_true, label_length):
    ext = np.full((B, S), BLANK, np.int32)
    ext[:, 1::2] = y_true.astype(np.int32)
    prev2 = np.concatenate([np.full((B, 2), BLANK, np.int32), ext[:, :-2]], axis=1)
    skip = ((ext != BLANK) & (ext != prev2)).astype(np.float32)
    skip2 = np.concatenate([skip[:, 2:], np.zeros((B, 2), np.float32)], axis=1)
    P = np.take_along_axis(
        np.ascontiguousarray(y_pred, dtype=np.float32), ext[:, None, :], axis=2
    )
    P += np.float32(EPS)
    L = label_length.reshape(B).astype(np.int64)
    i2 = np.clip(2 * L, 0, S - 1)
    i1 = np.maximum(i2 - 1, 0)
    # reachability mask: position s at time t is dead if it cannot reach i1
    # by t=T-1 (max +2 per step). Folded into P at zero device cost.
    s_idx = np.arange(S)[None, None, :]
    t_idx = np.arange(T)[None, :, None]
    alive = (s_idx + 2 * (T - 1 - t_idx)) >= i1[:, None, None]
    P *= alive.astype(np.float32)
    Ppad = np.zeros((B, T, SP), np.float32)
    Ppad[:, :, :S] = P
    skipp = np.zeros((B, SP), np.float32)
    skipp[:, :S] = skip
    skip2p = np.zeros((B, SP), np.float32)
    skip2p[:, :S] = skip2
    selp = np.zeros((B, SP), np.float32)
    selp[np.arange(B), i1] = 1.0
    selp[np.arange(B), i2] = 1.0
    in_maps = []
    for c in range(NCORES):
        sl = slice(c * BS, (c + 1) * BS)
        in_maps.append(
            {
                "p": np.ascontiguousarray(Ppad[sl]),
                "skip": np.ascontiguousarray(skipp[sl]),
                "skip2": np.ascontiguousarray(skip2p[sl]),
                "sel": np.ascontiguousarray(selp[sl]),
            }
        )
    return in_maps


def _run_device(in_maps, **kwargs):
    nc = _get_nc()
    return run_bass_kernel_spmd(nc, in_maps, core_ids=list(range(NCORES)), **kwargs)


def _ctc_numpy(y_pred, y_true, input_length, label_length):
    """Generality safety net (log domain, mirrors the reference exactly)."""
    b, t_max, c = y_pred.shape
    u = y_true.shape[1]
    s = 2 * u + 1
    blank = c - 1
    neg = np.float32(-1e30)
    logp = np.log(y_pred.astype(np.float32) + np.float32(EPS))
    ext = np.full((b, s), blank, np.int32)
    ext[:, 1::2] = y_true.astype(np.int32)
    prev2 = np.concatenate([np.full((b, 2), blank, np.int32), ext[:, :-2]], axis=1)
    can_skip = (ext != blank) & (ext != prev2)
    lp_ext = np.take_along_axis(logp, ext[:, None, :], axis=2)
    alpha = np.full((b, s), neg, np.float32)
    alpha[:, 0] = lp_ext[:, 0, 0]
    alpha[:, 1] = lp_ext[:, 0, 1]
    inp_len = input_length.reshape(b)

    def lse(stack):
        m = np.max(stack, axis=0)
        return m + np.log(np.sum(np.exp(stack - m), axis=0))

    for t in range(1, t_max):
        a1 = np.concatenate([np.full((b, 1), neg, np.float32), alpha[:, :-1]], axis=1)
        a2 = np.concatenate([np.full((b, 2), neg, np.float32), alpha[:, :-2]], axis=1)
        a2 = np.where(can_skip, a2, neg)
        new = lse(np.stack([alpha, a1, a2], 0)).astype(np.float32) + lp_ext[:, t, :]
        alpha = np.where((t < inp_len)[:, None], new, alpha)
    L = label_length.reshape(b).astype(np.int64)
    i2 = np.clip(2 * L, 0, s - 1)
    i1 = np.maximum(i2 - 1, 0)
    a_last = np.stack([alpha[np.arange(b), i1], alpha[np.arange(b), i2]], axis=1)
    ll = np.where(L > 0, lse(a_last.T).astype(np.float32), alpha[:, 0])
    return (-ll[:, None]).astype(np.float32)


def kernel(y_pred, y_true, input_length, label_length):
    y_pred = np.asarray(y_pred)
    y_true = np.asarray(y_true)
    input_length = np.asarray(input_length)
    label_length = np.asarray(label_length)
    if y_pred.shape != (B, T, C) or y_true.shape != (B, U) or not np.all(
        input_length.reshape(-1) == T
    ):
        return _ctc_numpy(y_pred, y_true, input_length, label_length)
    in_maps = _prep_in_maps(y_pred, y_true, label_length)
    res = _run_device(in_maps)
    out = np.concatenate([r["out"] for r in res.results], axis=0)
    return np.ascontiguousarray(out, dtype=np.float32)

